# revision 7
# baseline (speedup 1.0000x reference)
"""Bass/Tile TRN2 kernel: batch cosine contrastive loss (NxN cosine sim + CE diag).

Strategy (8-way data parallel over rows of output1):
  - each core gets: a_shard = output1[c*1024:(c+1)*1024]  (its row block)
                    b_full  = output2                      (replicated)
                    b_diag  = output2[c*1024:(c+1)*1024]   (for diagonal terms)
  - on device: cast bf16, row-normalize (DVE sumsq + ACT sqrt + DVE recip + DVE scale),
    store normalized bf16 rows to DRAM scratch, DMA-xbar-transpose back to K-major,
    bf16 matmul row-block x all-cols into PSUM, ACT Exp with fused row-sum accumulate,
    diagonal via elementwise TTR of normalized a/b_diag rows, Ln(rowsum) - diag.
  - host: mean over all 8192 per-row losses -> scalar float32.
"""

import os

import numpy as np

import concourse.bacc as bacc
import concourse.mybir as mybir
import concourse.tile as tile
from concourse import bass_utils

F32 = mybir.dt.float32
BF16 = mybir.dt.bfloat16
AluOp = mybir.AluOpType
Act = mybir.ActivationFunctionType

# problem constants (hardcoded per contract)
N, D = 8192, 256
NCORES = 8
SH = N // NCORES  # 1024 rows per core

LAST_RESULTS = None
_CACHE = {}
_HOOK_READY = False


def _install_ntff_hook():
    """Provide antenv.axon_hooks + disable artifact upload so trace=True works.

    The agent image's antenv package lacks axon_hooks; the NTFF profile
    mechanism itself (libaxon_pjrt.so C ABI) is present. Mirrors
    trn_agent_boot.trn_boot._ntff_profile_via_ctypes.
    """
    global _HOOK_READY
    if _HOOK_READY:
        return
    import contextlib
    import ctypes
    import sys
    import types

    bass_utils.upload_artifacts = lambda tmpdir: "local://skipped"

    try:
        from antenv.axon_hooks import get_axon_ntff_profile_hook  # noqa: F401

        _HOOK_READY = True
        return
    except ImportError:
        pass

    so_path = "/opt/axon/libaxon_pjrt.so"
    hook = None
    try:
        lib = ctypes.CDLL(so_path)
        if hasattr(lib, "axon_start_nrt_profile"):
            lib.axon_start_nrt_profile.argtypes = [
                ctypes.POINTER(ctypes.c_int64),
                ctypes.c_size_t,
            ]
            lib.axon_start_nrt_profile.restype = ctypes.c_int64
            lib.axon_stop_nrt_profile.argtypes = [ctypes.c_char_p]
            lib.axon_stop_nrt_profile.restype = ctypes.c_int64

            @contextlib.contextmanager
            def _hook(output_dir, device_ids):
                import jax

                jax.devices()
                if device_ids:
                    ids = (ctypes.c_int64 * len(device_ids))(*device_ids)
                    rc = lib.axon_start_nrt_profile(ids, len(device_ids))
                else:
                    rc = lib.axon_start_nrt_profile(None, 0)
                if rc != 0:
                    raise RuntimeError(f"axon_start_nrt_profile rc={rc}")
                try:
                    yield
                finally:
                    n = lib.axon_stop_nrt_profile(str(output_dir).encode())
                    print(f"ntff profile: {n} file(s) -> {output_dir}")

            hook = _hook
    except OSError:
        hook = None

    mod = types.ModuleType("antenv.axon_hooks")
    mod._hook = hook
    mod.get_axon_ntff_profile_hook = lambda: mod._hook
    mod.set_axon_ntff_profile_hook = lambda h: setattr(mod, "_hook", h)
    sys.modules["antenv.axon_hooks"] = mod
    _HOOK_READY = True


def build_program(n, sh, num_devices, nc_chunk=2048):
    """Build the per-core bass program. n: total B rows; sh: A-shard rows."""
    assert n % 1024 == 0 and sh % 128 == 0 and n % nc_chunk == 0
    mt = sh // 128          # m tiles (128 rows each)
    g = n // 1024           # B groups of 1024 rows
    kc = D // 128           # 2 contraction chunks
    nch = n // nc_chunk     # psum-tile column chunks
    nb = nc_chunk // 512    # matmuls of 512 cols per psum tile

    nc = bacc.Bacc(
        "TRN2",
        target_bir_lowering=False,
        debug=False,
        enable_asserts=False,
        num_devices=num_devices,
    )
    a_dram = nc.dram_tensor("a_shard", (sh, D), F32, kind="ExternalInput")
    b_dram = nc.dram_tensor("b_full", (n, D), F32, kind="ExternalInput")
    bd_dram = nc.dram_tensor("b_diag", (sh, D), F32, kind="ExternalInput")
    out_dram = nc.dram_tensor("loss_rows", (128, mt), F32, kind="ExternalOutput")
    scr_a = nc.dram_tensor("scr_a", (sh, D), BF16, kind="Internal")
    scr_b = nc.dram_tensor("scr_b", (n, D), BF16, kind="Internal")

    with tile.TileContext(nc) as tc:
        with (
            tc.tile_pool(name="persist", bufs=1) as pp,
            tc.tile_pool(name="ld", bufs=2) as ld,
            tc.tile_pool(name="bfp", bufs=2) as bfp,
            tc.tile_pool(name="small", bufs=3) as sm,
            tc.tile_pool(name="sqp", bufs=2) as sqp,
            tc.tile_pool(name="exp", bufs=2) as exq,
            tc.tile_pool(name="psum", bufs=2, space="PSUM") as psp,
        ):
            at = [
                pp.tile([128, sh], BF16, tag=f"at{k}", name=f"at{k}")
                for k in range(kc)
            ]
            bt = [
                [
                    pp.tile(
                        [128, 1024], BF16, tag=f"bt{k}_{gi}", name=f"bt{k}_{gi}"
                    )
                    for gi in range(g)
                ]
                for k in range(kc)
            ]
            rs_parts = pp.tile([128, mt, nch], F32, tag="rsp")
            diag = pp.tile([128, mt], F32, tag="diag")

            def normalize_rows(src_dram, row0, nrows, out_tile):
                """Load rows [row0, row0+nrows), normalize in bf16, write to out_tile.

                out_tile: [128, nrows//128, D] bf16; partition p holds row t*128+p.
                """
                nt = nrows // 128
                f32t = ld.tile([128, nt, D], F32, tag="ld")
                src = src_dram.ap()[row0 : row0 + nrows].rearrange(
                    "(t p) k -> p t k", p=128
                )
                nc.sync.dma_start(f32t[:], src)
                bf = bfp.tile([128, nt, D], BF16, tag="bf")
                nc.vector.tensor_copy(bf[:], f32t[:])
                ssq = sm.tile([128, nt], F32, tag="ssq")
                for t in range(nt):
                    prod = sqp.tile([128, D], F32, tag="sq")
                    nc.vector.scalar_tensor_tensor(
                        out=prod[:],
                        in0=bf[:, t],
                        scalar=1.0,
                        in1=bf[:, t],
                        op0=AluOp.mult,
                        op1=AluOp.mult,
                        accum_out=ssq[:, t : t + 1],
                    )
                nrm = sm.tile([128, nt], F32, tag="nrm")
                nc.scalar.sqrt(nrm[:], ssq[:])
                inv = sm.tile([128, nt], F32, tag="inv")
                nc.vector.reciprocal(inv[:], nrm[:])
                for t in range(nt):
                    nc.vector.tensor_scalar_mul(
                        out_tile[:, t], bf[:, t], inv[:, t : t + 1]
                    )

            # ---- A shard: normalize, store, transpose-load K-major ----
            a_nrm = pp.tile([128, mt, D], BF16, tag="anrm")
            normalize_rows(a_dram, 0, sh, a_nrm)
            dst = scr_a.ap().rearrange("(t p) k -> p t k", p=128)
            nc.sync.dma_start(dst, a_nrm[:])
            for k in range(kc):
                nc.sync.dma_start_transpose(
                    at[k][:], scr_a.ap()[:, k * 128 : (k + 1) * 128]
                )

            # ---- B full: per 1024-row group ----
            for gi in range(g):
                b_nrm = bfp.tile([128, 8, D], BF16, tag="nbf")
                normalize_rows(b_dram, gi * 1024, 1024, b_nrm)
                dstb = scr_b.ap()[gi * 1024 : (gi + 1) * 1024].rearrange(
                    "(t p) k -> p t k", p=128
                )
                nc.sync.dma_start(dstb, b_nrm[:])
                for k in range(kc):
                    nc.sync.dma_start_transpose(
                        bt[k][gi][:],
                        scr_b.ap()[
                            gi * 1024 : (gi + 1) * 1024, k * 128 : (k + 1) * 128
                        ],
                    )

            # ---- b_diag: normalize + diagonal terms ----
            bd_nrm = pp.tile([128, mt, D], BF16, tag="bdnrm")
            normalize_rows(bd_dram, 0, sh, bd_nrm)
            for t in range(mt):
                prod = sqp.tile([128, D], F32, tag="sq")
                nc.vector.scalar_tensor_tensor(
                    out=prod[:],
                    in0=a_nrm[:, t],
                    scalar=1.0,
                    in1=bd_nrm[:, t],
                    op0=AluOp.mult,
                    op1=AluOp.mult,
                    accum_out=diag[:, t : t + 1],
                )

            # ---- matmul + exp + row-sum ----
            for c in range(nch):
                for mi in range(mt):
                    ps = psp.tile([128, nc_chunk], F32, tag="ps")
                    for k in range(kc):
                        for b in range(nb):
                            col = c * nc_chunk + b * 512
                            gi, off = col // 1024, col % 1024
                            nc.tensor.matmul(
                                ps[:, b * 512 : (b + 1) * 512],
                                at[k][:, mi * 128 : (mi + 1) * 128],
                                bt[k][gi][:, off : off + 512],
                                start=(k == 0),
                                stop=(k == kc - 1),
                            )
                    ex = exq.tile([128, nc_chunk], BF16, tag="ex")
                    nc.scalar.activation(
                        ex[:],
                        ps[:],
                        Act.Exp,
                        accum_out=rs_parts[:, mi, c : c + 1],
                    )

            # ---- finalize: loss = ln(rowsum) - diag ----
            rowsum = sm.tile([128, mt], F32, tag="rs")
            nc.vector.tensor_reduce(
                out=rowsum[:], in_=rs_parts[:], axis=mybir.AxisListType.X, op=AluOp.add
            )
            logz = sm.tile([128, mt], F32, tag="logz")
            nc.scalar.activation(logz[:], rowsum[:], Act.Ln)
            lossr = sm.tile([128, mt], F32, tag="loss")
            nc.vector.tensor_tensor(
                out=lossr[:], in0=logz[:], in1=diag[:], op=AluOp.subtract
            )
            nc.sync.dma_start(out_dram.ap(), lossr[:])

    nc.compile()
    return nc


def _get_program():
    key = (N, SH, NCORES)
    if key not in _CACHE:
        _CACHE[key] = build_program(N, SH, NCORES)
    return _CACHE[key]


def kernel(output1: np.ndarray, output2: np.ndarray) -> np.ndarray:
    global LAST_RESULTS
    o1 = np.ascontiguousarray(np.asarray(output1, dtype=np.float32))
    o2 = np.ascontiguousarray(np.asarray(output2, dtype=np.float32))
    assert o1.shape == (N, D) and o2.shape == (N, D)

    trace = bool(int(os.environ.get("KERNEL_TRACE", "0")))
    if trace:
        _install_ntff_hook()
    nc = _get_program()
    in_maps = [
        {
            "a_shard": o1[c * SH : (c + 1) * SH],
            "b_full": o2,
            "b_diag": o2[c * SH : (c + 1) * SH],
        }
        for c in range(NCORES)
    ]
    res = bass_utils.run_bass_kernel_spmd(
        nc,
        in_maps,
        core_ids=list(range(NCORES)),
        trace=trace,
        tmpdir=os.environ.get("KERNEL_TRACE_DIR") or None,
    )
    LAST_RESULTS = res
    losses = np.concatenate(
        [r["loss_rows"].T.reshape(-1) for r in res.results]
    )  # loss_rows[p, t] is row t*128+p of the shard
    return np.asarray(losses.mean(dtype=np.float64), dtype=np.float32)
